# revision 54
# baseline (speedup 1.0000x reference)
"""3-layer GCN forward (GCNConv x3 + log_softmax) on 8 Trainium2 cores.

Strategy (self-contained; shapes hardcoded for N=100000, Cin=Ch=128,
Cout=47, 8 cores): A_hat = D^-1/2 (A+I) D^-1/2 fixed across layers, so
per layer out = dinv_dst * segsum_dst(dinv_src * (H @ W)) + b.

Host: permute nodes into 8 contiguous core blocks (degree-ranked
round-robin so all cores share one loop structure / NEFF). Self-loop
messages are dropped from the edge stream (added on-chip from an
SBUF-resident copy of the local Z block via an identity matmul).

Edge stream layout per core: window-major (4 source windows = (block
half, core quad) of 25088 rows each, int16-rebased), groups contiguous
inside each window at offsets given by the per-(group,window) max count
across cores (so all 8 cores share one instruction stream); a greedy
sum-of-squares bin-packing of dst nodes into groups flattens those
cells (pad ~1.4%). Message tiles of 128 may straddle two adjacent
groups; ids are stored per group copy as (rloc - 128g), so every SEL
matches a single iota window and builds in uniform 16-tile DVE batches.

The gather uses batched dma_gather (16-tile chunks = 2048 descriptors,
exactly two per SWDGE ring; 4 queues, one per window) from AllGathered
bf16 Z half-replicas. The first-half AllGather is issued mid-layer
(after group 48) to overlap the exchange; the second half's exposure is
hidden by pre-issuing the next layer's first-half gather descgen.
Aggregation per tile: matmul(lhsT=SEL, rhs=messages) into the group's
PSUM accumulator; + identity-matmul self-loop from the SBUF-resident
local Z; + rank-1 bias (binv x b) when present. Post per group: fused
scale+relu (scalar), PE transpose, next-layer GEMM, scale (DVE,
precision), zs write batched per 7 groups. Final layer: scale +
log_softmax fused per 7 groups under the remaining gathers.
"""
import numpy as np
import ml_dtypes

NCORES = 8
N = 100000
NBLK = 12500
NPAD = 12544            # 98 * 128
NGRP = NPAD // 128      # 98
C = 128
COUT = 47
HALFA = 6272            # rows in first half block (49 groups)
HALFB = NPAD - HALFA    # 6272 rows (49 groups)
GA = HALFA // 128       # 63
QWIN = (4 * HALFA, 4 * HALFB)   # rows per source window by half
NQ = 4                  # windows: w = 2*half + corequad
CHUNK_T = 16            # tiles per gather instruction (2 instr fit the ring)
GP_BUFS = 3             # gather pool depth per quarter
LOOKAHEAD_G = 2         # groups of gather prefetch (window <= GP_BUFS)
PADROW = 0              # any valid row; pad slots excluded via ids
PADID = 512.0           # ids value matching no iota window


def _preprocess(x, edge_index, W1, b1, W2, b2, W3, b3):
    x = np.asarray(x, np.float32)
    ei = np.asarray(edge_index)
    src_e = ei[0].astype(np.int64)
    dst_e = ei[1].astype(np.int64)
    nE = src_e.shape[0]

    deg = (np.bincount(dst_e, minlength=N) + 1).astype(np.float32)
    dinv = 1.0 / np.sqrt(deg)

    rank = np.argsort(-deg, kind="stable")
    perm = np.empty(N, np.int64)
    for k in range(NCORES):
        perm[k * NBLK:(k + 1) * NBLK] = rank[k::NCORES]
    inv = np.empty(N, np.int64)
    inv[perm] = np.arange(N)

    # rebalance groups within each (core, half): flatten per-(group,
    # window) in-edge cell counts so the cross-core max (which sets the
    # shared stream layout) stays near the mean. Window membership of
    # sources is untouched by within-half dst moves.
    srcp0 = inv[src_e]
    ks0 = srcp0 // NBLK
    r0 = srcp0 - ks0 * NBLK
    w0 = 2 * (r0 >= HALFA).astype(np.int64) + ks0 // 4
    # per ORIGINAL dst node: in-edge count by source window
    node_w = np.bincount(dst_e * NQ + w0, minlength=N * NQ) \
        .reshape(N, NQ).astype(np.float64)
    for k in range(NCORES):
        for h in range(2):
            lo = k * NBLK + h * HALFA
            hi = min(lo + (HALFA if h == 0 else HALFB), (k + 1) * NBLK)
            GH = GA if h == 0 else NGRP - GA
            M = hi - lo
            dw = node_w[perm[lo:hi]]
            order_n = np.argsort(-dw.sum(1), kind="stable")
            bins = np.zeros((GH, NQ))
            cnt = np.zeros(GH, np.int64)
            caps = np.full(GH, 128, np.int64)
            if M < GH * 128:                      # shortfall -> last group
                caps[-1] = M - (GH - 1) * 128
            asg = np.empty(M, np.int64)
            for n in order_n:
                d = dw[n]
                score = (bins @ d) + 0.5 * (bins * bins).sum(1)
                score[cnt >= caps] = 1e18
                b = int(np.argmin(score))
                asg[n] = b
                bins[b] += d
                cnt[b] += 1
            new_order = np.argsort(asg * M + np.arange(M), kind="stable")
            perm[lo:hi] = perm[lo:hi][new_order]
    inv[perm] = np.arange(N)

    srcp = inv[src_e]
    dstp = inv[dst_e]
    ksrc = srcp // NBLK
    srcg = ksrc * NPAD + (srcp - ksrc * NBLK)     # padded-global coords
    dinv_p = dinv[perm]

    ecore = dstp // NBLK
    rloc = dstp - ecore * NBLK
    grp = rloc // 128
    # source window: (half of source block, quad of source cores)
    ks = srcg // NPAD
    rrow = srcg - ks * NPAD
    hh = (rrow >= HALFA).astype(np.int64)
    hsz = np.where(hh == 0, HALFA, HALFB)
    widx = (ks % 4) * hsz + (rrow - hh * HALFA)   # within-window row
    qq = 2 * hh + ks // 4

    # shared stream layout: per (group, quarter) slot count = max over cores
    key = (ecore * NGRP + grp) * NQ + qq
    cnt = np.bincount(key, minlength=NCORES * NGRP * NQ) \
        .reshape(NCORES, NGRP, NQ)
    cmax = cnt.max(axis=0)                        # [NGRP, NQ]
    assert cmax.min() >= 128, "tile may straddle >2 groups"
    off = np.zeros((NGRP + 1, NQ), np.int64)
    off[1:] = np.cumsum(cmax, axis=0)
    Lq = off[NGRP]                                # slots per quarter
    Tq = (Lq + 127) // 128                        # tiles per quarter
    qtile_off = np.zeros(NQ + 1, np.int64)
    qtile_off[1:] = np.cumsum(Tq)
    Ttot = int(qtile_off[-1])

    # edge order (core, quarter, group, src) -> ascending HBM walk per seg
    order = np.lexsort((widx, grp, qq, ecore))
    e_core = ecore[order]
    e_q = qq[order]
    e_g = grp[order]
    e_widx = widx[order]
    e_rloc = rloc[order]
    run_key = (e_core * NQ + e_q) * NGRP + e_g
    cnt_run = np.bincount(run_key, minlength=NCORES * NQ * NGRP)
    run_starts = np.zeros(NCORES * NQ * NGRP + 1, np.int64)
    np.cumsum(cnt_run, out=run_starts[1:])
    within = np.arange(nE) - run_starts[run_key]
    slot = off[e_g, e_q] + within                 # slot in quarter stream

    idx16 = np.full((NCORES, Ttot * 128), PADROW, np.int16)
    ids = np.full((NCORES, Ttot * 128), -1e9, np.float32)
    gpos = qtile_off[e_q] * 128 + slot
    idx16[e_core, gpos] = e_widx.astype(np.int16)
    ids[e_core, gpos] = e_rloc.astype(np.float32)
    for q in range(NQ):                           # trailing pads: skipped
        idx16[:, qtile_off[q] * 128 + Lq[q]:qtile_off[q + 1] * 128] = -1

    # group tile spans (quarter-local tiles, inclusive)
    tlo = off[:-1] // 128                         # [NGRP, NQ]
    thi = (off[1:] - 1) // 128
    ntg_q = thi - tlo + 1
    ntg = ntg_q.sum(axis=1)
    gm_base = np.zeros(NGRP + 1, np.int64)
    gm_base[1:] = np.cumsum(ntg)
    GM = int(gm_base[-1])
    group_tiles = []
    tile_cols = []                                # global tile idx per GM col
    col_grp = []                                  # owning group per GM col
    for g in range(NGRP):
        lst = []
        for q in range(NQ):
            for t in range(tlo[g, q], thi[g, q] + 1):
                lst.append((int(q), int(t)))
                tile_cols.append(int(qtile_off[q] + t))
                col_grp.append(g)
        group_tiles.append(lst)
    tile_cols = np.array(tile_cols, np.int64)
    col_grp = np.array(col_grp, np.int64)
    c_need = (thi // CHUNK_T)                     # [NGRP, NQ] last chunk
    nchunks = [int((Tq[q] + CHUNK_T - 1) // CHUNK_T) for q in range(NQ)]
    chunk_nt = [[min(CHUNK_T, int(Tq[q]) - c * CHUNK_T)
                 for c in range(nchunks[q])] for q in range(NQ)]

    # SBUF layouts
    idxw = Ttot * 8
    w16 = idx16.reshape(NCORES, Ttot * 8, 16).transpose(0, 2, 1)
    idx_sb = np.tile(w16, (1, 8, 1))              # [8, 128, idxw]
    ids_t = ids.reshape(NCORES, Ttot, 128)
    # per-group-copy rebased ids: group g matches iota 0..127 directly
    vals = ids_t[:, tile_cols, :]                 # [8, GM, 128]
    adj = vals - (col_grp[None, :, None] * 128.0)
    ids_gm = np.where(vals < -1e8, PADID, adj).transpose(0, 2, 1)

    dinv_loc = np.zeros((NCORES, 128, NGRP), np.float32)
    binv_row = np.zeros((NCORES, 1, NPAD), np.float32)
    dv = dinv_p.reshape(NCORES, NBLK)
    for k in range(NCORES):
        full = np.zeros(NPAD, np.float32)
        full[:NBLK] = dv[k]
        dinv_loc[k] = full.reshape(NGRP, 128).T
        with np.errstate(divide="ignore"):
            binv_row[k, 0] = np.where(full > 0, 1.0 / full, 0.0)

    # fold dinv into x rows: (dinv*x) @ W1 == dinv * (x @ W1); removes the
    # per-group scale from the prologue pipeline
    xp = x[perm] * dinv_p[:, None]
    xblkT = np.zeros((NCORES, C, NPAD), np.float32)
    for k in range(NCORES):
        xblkT[k, :, :NBLK] = xp[k * NBLK:(k + 1) * NBLK].T

    Ws = [np.ascontiguousarray(W, np.float32) for W in (W1, W2, W3)]
    brows = [np.asarray(b, ml_dtypes.bfloat16).reshape(1, -1)
             for b in (b1, b2, b3)]
    has_bias = any(np.any(np.asarray(b) != 0) for b in (b1, b2, b3))
    iota = np.arange(128, dtype=np.float32)
    iota2 = np.tile(iota[None, None, :], (128, 16, 1)) \
        .reshape(128, 16 * 128).astype(ml_dtypes.bfloat16)

    in_maps = []
    for k in range(NCORES):
        m = {
            "xblkT": np.ascontiguousarray(xblkT[k]),
            "gidx": np.ascontiguousarray(idx_sb[k]),
            "gids": np.ascontiguousarray(
                ids_gm[k].astype(ml_dtypes.bfloat16)),
            "dinv": np.ascontiguousarray(dinv_loc[k]),
            "iota2": iota2,
            "w1": Ws[0], "w2": Ws[1], "w3": Ws[2],
        }
        if has_bias:
            m["binv"] = np.ascontiguousarray(
                binv_row[k].astype(ml_dtypes.bfloat16))
            m["br1"], m["br2"], m["br3"] = brows
        in_maps.append(m)
    meta = {
        "Ttot": Ttot, "idxw": idxw, "GM": GM,
        "qtile_off": qtile_off.tolist(),
        "nchunks": nchunks, "chunk_nt": chunk_nt,
        "group_tiles": group_tiles,
        "gm_base": gm_base.tolist(),
        "c_need": c_need.tolist(),
        "has_bias": bool(has_bias),
    }
    return in_maps, meta, perm


def _build(meta):
    from concourse import bacc, bass, mybir, tile
    from concourse.masks import make_identity
    f32 = mybir.dt.float32
    bf16 = mybir.dt.bfloat16
    i16 = mybir.dt.int16
    AF = mybir.ActivationFunctionType

    idxw = meta["idxw"]
    GM = meta["GM"]
    qtile_off = meta["qtile_off"]
    nchunks = meta["nchunks"]
    chunk_nt = meta["chunk_nt"]
    group_tiles = meta["group_tiles"]
    gm_base = meta["gm_base"]
    c_need = meta["c_need"]
    has_bias = meta["has_bias"]

    nc = bacc.Bacc("TRN2", target_bir_lowering=False, debug=False,
                   num_devices=NCORES, num_swdge_queues=4)
    xTd = nc.dram_tensor("xblkT", [C, NPAD], f32, kind="ExternalInput")
    gidx = nc.dram_tensor("gidx", [128, idxw], i16, kind="ExternalInput")
    gids = nc.dram_tensor("gids", [128, GM], bf16, kind="ExternalInput")
    dinv = nc.dram_tensor("dinv", [128, NGRP], f32, kind="ExternalInput")
    iota_in = nc.dram_tensor("iota2", [128, 16 * 128], bf16,
                             kind="ExternalInput")
    w_in = [nc.dram_tensor(f"w{l+1}", [C, co], f32, kind="ExternalInput")
            for l, co in enumerate([C, C, COUT])]
    if has_bias:
        binv = nc.dram_tensor("binv", [1, NPAD], bf16, kind="ExternalInput")
        br_in = [nc.dram_tensor(f"br{l+1}", [1, co], bf16,
                                kind="ExternalInput")
                 for l, co in enumerate([C, C, COUT])]
    out_d = nc.dram_tensor("out", [NPAD, COUT], f32, kind="ExternalOutput")

    zs = [nc.dram_tensor(f"zs{l}", [NPAD, C], bf16) for l in range(3)]
    # AllGathered halves: zfh[h][l] holds the A rows [0,HALFA) or B rows
    # [HALFA,NPAD) of all cores' zs[l]; A (63 groups) is issued mid-layer
    # so the exchange overlaps the aggregation tail; B (35 groups) is
    # small enough to hide behind the next layer's first-half descgen.
    hsizes = (HALFA, HALFB)
    zfh = [[nc.dram_tensor(f"zf{h}_{l}", [NCORES * hsizes[h], C], bf16,
                           addr_space="Shared") for l in range(3)]
           for h in range(2)]

    with tile.TileContext(nc) as tc:
        with tc.tile_pool(name="const", bufs=1) as cpool, \
             tc.tile_pool(name="g0", bufs=GP_BUFS) as gp0, \
             tc.tile_pool(name="g1", bufs=GP_BUFS) as gp1, \
             tc.tile_pool(name="g2", bufs=GP_BUFS) as gp2, \
             tc.tile_pool(name="g3", bufs=GP_BUFS) as gp3, \
             tc.tile_pool(name="sel", bufs=4) as selpool, \
             tc.tile_pool(name="work", bufs=4) as wpool, \
             tc.tile_pool(name="smp", bufs=8) as smp, \
             tc.tile_pool(name="ps_g", bufs=2, space="PSUM") as ps_g, \
             tc.tile_pool(name="ps_t", bufs=2, space="PSUM") as ps_t, \
             tc.tile_pool(name="ps_z", bufs=2, space="PSUM") as ps_z:
            gpools = [gp0, gp1, gp2, gp3]

            ident = cpool.tile([128, 128], f32)
            make_identity(nc, ident[:])
            identb = cpool.tile([128, 128], bf16)
            make_identity(nc, identb[:])
            iota_t = cpool.tile([128, 16, 128], bf16)
            nc.sync.dma_start(out=iota_t[:], in_=iota_in[:])
            idx_sb = cpool.tile([128, idxw], i16)
            nc.sync.dma_start(out=idx_sb[:], in_=gidx[:])
            ids_sb = cpool.tile([128, GM], bf16)
            nc.sync.dma_start(out=ids_sb[:], in_=gids[:])
            dinv_sb = cpool.tile([128, NGRP], f32)
            nc.sync.dma_start(out=dinv_sb[:], in_=dinv[:])
            zself = cpool.tile([128, NGRP, C], bf16)
            w_sb, br_sb = [], []
            for l, co in enumerate([C, C, COUT]):
                w = cpool.tile([128, co], f32, name=f"w_sb{l}")
                nc.sync.dma_start(out=w[:], in_=w_in[l][:])
                w_sb.append(w)
            if has_bias:
                binv_sb = cpool.tile([1, NPAD], bf16)
                nc.sync.dma_start(out=binv_sb[:], in_=binv[:])
                for l, co in enumerate([C, C, COUT]):
                    bt = cpool.tile([1, co], bf16, name=f"br_sb{l}")
                    nc.sync.dma_start(out=bt[:], in_=br_in[l][:])
                    br_sb.append(bt)

            # SEL batches are layer-independent (ids const); the dict is
            # cleared per layer (pool rotation) but prebuilt ones warm the
            # prologue's idle DVE
            sel_bufs = {}
            nselb = (GM + 15) // 16

            def ensure_sel(bn):
                while len(sel_bufs) <= bn:
                    b = len(sel_bufs)
                    w16v = min(16, GM - b * 16)
                    selt = selpool.tile([128, 16, 128], bf16, name="sel")
                    nc.vector.tensor_tensor(
                        out=selt[:, :w16v, :],
                        in0=iota_t[:, :w16v, :],
                        in1=ids_sb[:, b * 16:b * 16 + w16v]
                            .to_broadcast([128, w16v, 128]),
                        op=mybir.AluOpType.is_equal)
                    sel_bufs[b] = selt

            ensure_sel(3)

            # ---- layer-1 GEMM: zself/zs0 = dinv * (x @ W1), cast bf16 ----
            XB = 7
            for g0 in range(0, NGRP, XB):
                nb = min(XB, NGRP - g0)
                xt = wpool.tile([128, XB, 128], f32, name="xt")
                nc.sync.dma_start(
                    out=xt[:, :nb, :],
                    in_=xTd[:, g0 * 128:(g0 + nb) * 128])
                for g in range(g0, g0 + nb):
                    psz = ps_z.tile([128, C], f32, name="psz")
                    nc.tensor.matmul(out=psz[:], lhsT=xt[:, g - g0, :],
                                     rhs=w_sb[0][:], start=True, stop=True)
                    nc.scalar.activation(out=zself[:, g, :], in_=psz[:],
                                         func=AF.Copy)
                nc.sync.dma_start(
                    out=zs[0][g0 * 128:(g0 + nb) * 128, :]
                        .rearrange("(gg p) c -> p gg c", p=128),
                    in_=zself[:, g0:g0 + nb, :])
                if g0 + nb == GA:
                    nc.gpsimd.collective_compute(
                        "AllGather", mybir.AluOpType.bypass,
                        replica_groups=[list(range(NCORES))],
                        ins=[zs[0][0:HALFA, :]], outs=[zfh[0][0][:, :]])
            # per-layer gather state, hoisted so a layer's first-half
            # chunks can be issued BEFORE the previous layer's trailing
            # collective (otherwise they serialize behind it)
            chunk_bufs_all = [[dict() for _ in range(NQ)] for _ in range(3)]
            next_c_all = [[0] * NQ for _ in range(3)]

            def ensure(lay, q, cn):
                chunk_bufs = chunk_bufs_all[lay]
                next_c = next_c_all[lay]
                while next_c[q] <= cn:
                    cc = next_c[q]
                    nt = chunk_nt[q][cc]
                    buf = gpools[q].tile([128, CHUNK_T, C], bf16,
                                         name=f"gb{q}")
                    t0 = qtile_off[q] + cc * CHUNK_T
                    src = zfh[q // 2][lay]
                    j = q % 2
                    qr = QWIN[q // 2]
                    nc.gpsimd.dma_gather(
                        buf[:, :nt, :],
                        src[j * qr:(j + 1) * qr, :],
                        idx_sb[:, t0 * 8:(t0 + nt) * 8],
                        nt * 128, nt * 128, C, queue_num=q,
                        single_packet=False)
                    chunk_bufs[q][cc] = buf
                    next_c[q] += 1

            # prologue: B-collective first (its trigger must not queue
            # behind gathers that stall on A), then pre-issue layer-0
            # first-half chunks
            nc.gpsimd.collective_compute(
                "AllGather", mybir.AluOpType.bypass,
                replica_groups=[list(range(NCORES))],
                ins=[zs[0][HALFA:, :]], outs=[zfh[1][0][:, :]])
            for q in (0, 1):
                ensure(0, q, min(GP_BUFS - 1, nchunks[q] - 1))

            # ---- per layer: stream-gather aggregation (+ GEMM fusion) ----
            for lay in range(3):
                chunk_bufs = chunk_bufs_all[lay]

                if lay > 0:
                    sel_bufs.clear()

                for g in range(NGRP):
                    ga = min(g + LOOKAHEAD_G, NGRP - 1)
                    for q in range(NQ):
                        ensure(lay, q, c_need[ga][q])
                    base = gm_base[g]
                    gt = group_tiles[g]
                    ntgg = len(gt)
                    ensure_sel(min((base + ntgg) // 16, nselb - 1))
                    psg = ps_g.tile([128, C], f32, name="psg")
                    for jg, (q, t) in enumerate(gt):
                        buf = chunk_bufs[q][t // CHUNK_T]
                        jc = base + jg
                        nc.tensor.matmul(
                            out=psg[:],
                            lhsT=sel_bufs[jc // 16][:, jc % 16, :],
                            rhs=buf[:, t % CHUNK_T, :],
                            start=(jg == 0), stop=False)
                    # self-loop: psg += zself[g]  (dinv_src already folded)
                    nc.tensor.matmul(out=psg[:], lhsT=identb[:],
                                     rhs=zself[:, g, :], start=False,
                                     stop=(not has_bias))
                    co = C if lay < 2 else COUT
                    if has_bias:
                        nc.tensor.matmul(
                            out=psg[:, :co],
                            lhsT=binv_sb[:, g * 128:(g + 1) * 128],
                            rhs=br_sb[lay][:], start=False, stop=True)
                    if lay < 2:
                        h = wpool.tile([128, 128], f32, name="h")
                        nc.scalar.activation(
                            out=h[:], in_=psg[:], func=AF.Relu,
                            scale=dinv_sb[:, g:g + 1])
                        pst = ps_t.tile([128, 128], f32, name="pst")
                        nc.tensor.transpose(out=pst[:], in_=h[:],
                                            identity=ident[:])
                        ht = wpool.tile([128, 128], f32, name="ht")
                        nc.scalar.activation(out=ht[:], in_=pst[:],
                                             func=AF.Copy)
                        co2 = C if lay == 0 else COUT
                        psz = ps_z.tile([128, C], f32, name="psz2")
                        nc.tensor.matmul(out=psz[:, :co2], lhsT=ht[:],
                                         rhs=w_sb[lay + 1][:],
                                         start=True, stop=True)
                        nc.vector.tensor_scalar_mul(
                            out=zself[:, g, :co2], in0=psz[:, :co2],
                            scalar1=dinv_sb[:, g:g + 1])
                        if g % 7 == 6:
                            g0b = g - 6
                            nc.sync.dma_start(
                                out=zs[lay + 1][g0b * 128:(g + 1) * 128, :co2]
                                    .rearrange("(gg p) c -> p gg c", p=128),
                                in_=zself[:, g0b:g + 1, :co2])
                        if g == GA - 1:
                            nc.gpsimd.collective_compute(
                                "AllGather", mybir.AluOpType.bypass,
                                replica_groups=[list(range(NCORES))],
                                ins=[zs[lay + 1][0:HALFA, :]],
                                outs=[zfh[0][lay + 1][:, :]])
                    else:
                        # log_softmax + output write batched per 7 groups;
                        # the tail work rides under the remaining gathers
                        if g % 7 == 0:
                            sm7 = smp.tile([128, 7, COUT], f32, name="sm7")
                        nc.vector.tensor_scalar_mul(
                            out=sm7[:, g % 7, :], in0=psg[:, :COUT],
                            scalar1=dinv_sb[:, g:g + 1])
                        if g % 7 == 6:
                            g0b = g - 6
                            mx7 = smp.tile([128, 7], f32, name="mx7")
                            nc.vector.tensor_reduce(
                                out=mx7[:], in_=sm7[:],
                                axis=mybir.AxisListType.X,
                                op=mybir.AluOpType.max)
                            nc.vector.tensor_tensor(
                                out=sm7[:], in0=sm7[:],
                                in1=mx7[:].to_broadcast([128, 7, COUT]),
                                op=mybir.AluOpType.subtract)
                            ex7 = smp.tile([128, 7, COUT], f32, name="ex7")
                            nc.scalar.activation(out=ex7[:], in_=sm7[:],
                                                 func=AF.Exp)
                            ss7 = smp.tile([128, 7], f32, name="ss7")
                            nc.vector.tensor_reduce(
                                out=ss7[:], in_=ex7[:],
                                axis=mybir.AxisListType.X,
                                op=mybir.AluOpType.add)
                            nc.scalar.activation(out=ss7[:], in_=ss7[:],
                                                 func=AF.Ln)
                            nc.vector.tensor_tensor(
                                out=sm7[:], in0=sm7[:],
                                in1=ss7[:].to_broadcast([128, 7, COUT]),
                                op=mybir.AluOpType.subtract)
                            nc.sync.dma_start(
                                out=out_d[g0b * 128:(g + 1) * 128, :]
                                    .rearrange("(gg p) j -> p gg j", p=128),
                                in_=sm7[:])
                if lay < 2:
                    # next layer's first-half chunks BEFORE the trailing
                    # collective: issued after it they serialize behind
                    # its completion even though they only need the
                    # (long-finished) first-half exchange
                    for q in (0, 1):
                        ensure(lay + 1, q, min(GP_BUFS - 1, nchunks[q] - 1))
                    nc.gpsimd.collective_compute(
                        "AllGather", mybir.AluOpType.bypass,
                        replica_groups=[list(range(NCORES))],
                        ins=[zs[lay + 1][HALFA:, :]],
                        outs=[zfh[1][lay + 1][:, :]])

    nc.compile()
    return nc


LAST_RES = None


def kernel(x, edge_index, W1, b1, W2, b2, W3, b3):
    import os
    from concourse.bass_utils import run_bass_kernel_spmd

    in_maps, meta, perm = _preprocess(
        x, edge_index, W1, b1, W2, b2, W3, b3)
    nc = _build(meta)
    kw = {}
    if os.environ.get("KERNEL_TRACE", "0") == "1":
        kw["trace"] = True
        if os.environ.get("KERNEL_TMPDIR"):
            kw["tmpdir"] = os.environ["KERNEL_TMPDIR"]
    res = run_bass_kernel_spmd(nc, in_maps, core_ids=list(range(NCORES)), **kw)
    global LAST_RES
    LAST_RES = res
    blocks = [res.results[k]["out"][:NBLK] for k in range(NCORES)]
    outp = np.concatenate(blocks, axis=0)
    out = np.empty((N, COUT), np.float32)
    out[perm] = outp
    return out
